# revision 43
# baseline (speedup 1.0000x reference)
"""Axial sigmoid-attention Trainium2 kernel (8 NeuronCores, SPMD) — v6.

Sharding: core = b*4 + axis*2 + half; each core runs ONE axis over half the
non-attended spatial extent of one batch element (2048 positions = 32 outer
x 64 attended). Host gathers and sums the two axes' contributions.

RoPE decomposition (no cross-partition ops): logits accumulate in PSUM as
  A: (c_t q)·(c_s k) + (s_t q)·(s_s k)
  B: (c_t q)·(±s_s ksw) + (s_t q)·(∓c_s ksw),  ksw = x @ Wk_swapneg (host).

v6 vs v5: engine rebalance (k-rotation products fused into 6 gpsimd ops/blk
with sign-baked tables; psum->sbuf casts split scalar/vector), consolidated
DMAs issued from 4 engines, one batched output DMA per block, 2 tile pools
(tag-rings) instead of 7 to cut barrier cost, slimmer PE warmup.
"""

import numpy as np

B, Y, X, C = 2, 64, 64, 256
M, KG = 4, 2
H, HV = 32, 32
HH = H // 2
SCALE = 1.0 / np.sqrt(H)
DEN = 1.0 / np.sqrt(65.0)
NPOS = 2048
NBLK = 4
BLK = NPOS // NBLK


# ---------------------------------------------------------------- bass program
def build_program():
    import concourse.bacc as bacc
    import concourse.mybir as mybir
    from concourse.tile import TileContext

    dt = mybir.dt
    AF = mybir.ActivationFunctionType

    nc = bacc.Bacc()

    xT = nc.declare_dram_parameter("xT", [256, NPOS], dt.bfloat16, isOutput=False)
    wq = nc.declare_dram_parameter("wq", [256, 1024], dt.bfloat16, isOutput=False)
    wk = nc.declare_dram_parameter("wk", [256, 512], dt.bfloat16, isOutput=False)
    wv = nc.declare_dram_parameter("wv", [256, 256], dt.bfloat16, isOutput=False)
    wo = nc.declare_dram_parameter("wo", [128, 8 * 256], dt.bfloat16, isOutput=False)
    ctab = nc.declare_dram_parameter("ctab", [128, 1024], dt.bfloat16, isOutput=False)
    stab = nc.declare_dram_parameter("stab", [128, 1024], dt.bfloat16, isOutput=False)
    outT = nc.declare_dram_parameter("outT", [256, NPOS], dt.bfloat16, isOutput=True)

    with TileContext(nc) as tc:
        with (
            tc.tile_pool(name="sb", bufs=1) as sb,
            tc.tile_pool(name="ps", bufs=1, space="PSUM") as ps,
        ):
            xt_sb = sb.tile([128, 2, NPOS], dt.bfloat16, tag="xt")
            wq_sb = sb.tile([128, 2, 1024], dt.bfloat16, tag="wq")
            wk_sb = sb.tile([128, 2, 512], dt.bfloat16, tag="wk")
            wv_sb = sb.tile([128, 2, 256], dt.bfloat16, tag="wv")
            wo_sb = sb.tile([128, 8, 256], dt.bfloat16, tag="wo")
            ct_sb = sb.tile([128, 1024], dt.bfloat16, tag="ct")
            st_sb = sb.tile([128, 1024], dt.bfloat16, tag="st")

            # input DMAs: ONLY sync+scalar have hardware DMA queues
            # (~128 B/ns each; gpsimd's software queue crawls at ~1 B/ns).
            # Per-queue FIFO order = criticality order: k-proj inputs, q-proj,
            # rope tables, v, remaining x blocks, out-proj weights.
            junk = sb.tile([128, 256], dt.bfloat16, tag="junk")
            nc.vector.memset(junk[:], 0.0)
            nc.sync.dma_start(out=xt_sb[:, 0, 0:BLK], in_=xT[0:128, 0:BLK])
            nc.scalar.dma_start(out=xt_sb[:, 1, 0:BLK], in_=xT[128:256, 0:BLK])
            nc.sync.dma_start(out=wk_sb[:, 0], in_=wk[0:128])
            nc.scalar.dma_start(out=wk_sb[:, 1], in_=wk[128:256])
            nc.sync.dma_start(out=wq_sb[:, 0], in_=wq[0:128])
            nc.scalar.dma_start(out=wq_sb[:, 1], in_=wq[128:256])
            nc.sync.dma_start(out=ct_sb[:], in_=ctab[:])
            nc.scalar.dma_start(out=st_sb[:], in_=stab[:])
            nc.sync.dma_start(out=wv_sb[:, 0], in_=wv[0:128])
            nc.scalar.dma_start(out=wv_sb[:, 1], in_=wv[128:256])
            for b in range(1, NBLK):
                nc.sync.dma_start(out=xt_sb[:, 0, b * BLK : (b + 1) * BLK],
                                  in_=xT[0:128, b * BLK : (b + 1) * BLK])
                nc.scalar.dma_start(out=xt_sb[:, 1, b * BLK : (b + 1) * BLK],
                                    in_=xT[128:256, b * BLK : (b + 1) * BLK])
            nc.sync.dma_start(out=wo_sb[:], in_=wo[:].rearrange("p (c n) -> p c n", c=8))

            # vector-clock warmups: each engine observes every input-DMA lane
            # via ops that depend on exactly one DMA (HW allows one sync wait
            # per instruction).
            warm = ps.tile([128, 512], dt.float32, tag="proj1", name="warm")
            _wi = [0]

            def warm_touch(sl):
                i = _wi[0]; _wi[0] += 1
                nc.tensor.matmul(warm[0:8, 8 * i : 8 * i + 8], lhsT=sl, rhs=sl,
                                 start=True, stop=True)

            # keep PE busy (and HAM warm) while input DMAs land: dependency-free
            # junk matmuls on a memset scratch tile
            jp = ps.tile([128, 512], dt.float32, tag="proj0", name="junkps")
            for i in range(8):
                nc.tensor.matmul(jp[0:128, 0:256], lhsT=junk[:, 0:128], rhs=junk[:, 0:256],
                                 start=True, stop=True)
            # stage 1: only what the first k-units need
            for sl in [wk_sb[:, 0, 0:8], wk_sb[:, 1, 0:8],
                       xt_sb[:, 0, 0:8], xt_sb[:, 1, 0:8]]:
                warm_touch(sl)
            wscr = sb.tile([128, 48], dt.bfloat16, tag="wscr")
            nc.vector.tensor_copy(wscr[:, 0:8], ct_sb[:, 0:8])
            nc.vector.tensor_copy(wscr[:, 8:16], st_sb[:, 0:8])
            nc.gpsimd.tensor_copy(wscr[:, 24:32], ct_sb[:, 0:8])
            nc.gpsimd.tensor_copy(wscr[:, 32:40], st_sb[:, 0:8])

            S = {}  # per-block live tiles

            def proj_units(blk):
                """Generator of emit-callbacks for block `blk` projections."""
                p0 = blk * BLK
                r = blk % 2
                st = S[blk] = {}
                st["q"] = [sb.tile([128, M * BLK], dt.bfloat16, tag=f"q{c}_{r}", name=f"q{c}_{blk}") for c in range(2)]
                st["cq"] = [sb.tile([128, M * BLK], dt.bfloat16, tag=f"cq{c}_{r}", name=f"cq{c}_{blk}") for c in range(2)]
                st["sq"] = [sb.tile([128, M * BLK], dt.bfloat16, tag=f"sq{c}_{r}", name=f"sq{c}_{blk}") for c in range(2)]
                st["ke"] = sb.tile([128, 4, BLK], dt.bfloat16, tag=f"ke_{r}", name=f"ke_{blk}")
                st["rk"] = [sb.tile([128, BLK], dt.bfloat16, tag=f"rk{c}_{r}", name=f"rk{c}_{blk}") for c in range(2)]
                st["rpk"] = [sb.tile([128, BLK], dt.bfloat16, tag=f"rpk{c}_{r}", name=f"rpk{c}_{blk}") for c in range(2)]
                st["vt"] = [sb.tile([128, 256], dt.bfloat16, tag=f"vt{i}_{r}", name=f"vt{i}_{blk}") for i in range(4)]
                st["vc"] = [sb.tile([128, 8 * 256], dt.bfloat16, tag=f"vc{q}_{r}", name=f"vc{q}_{blk}") for q in range(2)]

                def q_unit(m, cht):
                    def emit():
                        psq = ps.tile([128, BLK], dt.float32, tag=f"proj{m % 2}", name=f"qp{m}{cht}_{blk}")
                        for cc in range(2):
                            nc.tensor.matmul(
                                psq[:],
                                lhsT=wq_sb[:, cc, m * 256 + cht * 128 : m * 256 + (cht + 1) * 128],
                                rhs=xt_sb[:, cc, p0 : p0 + BLK],
                                start=(cc == 0), stop=(cc == 1),
                            )
                        if cht == 0 and m < 2:
                            nc.vector.tensor_copy(st["q"][cht][:, m * BLK : (m + 1) * BLK], psq[:])
                        else:
                            nc.scalar.copy(st["q"][cht][:, m * BLK : (m + 1) * BLK], psq[:])
                    return emit

                def k_unit(cht):
                    def emit():
                        psk = ps.tile([128, BLK], dt.float32, tag=f"proj{cht % 2}", name=f"kp{cht}_{blk}")
                        for cc in range(2):
                            nc.tensor.matmul(
                                psk[:],
                                lhsT=wk_sb[:, cc, cht * 128 : (cht + 1) * 128],
                                rhs=xt_sb[:, cc, p0 : p0 + BLK],
                                start=(cc == 0), stop=(cc == 1),
                            )
                        nc.scalar.copy(st["ke"][:, cht, :], psk[:])
                    return emit

                def vt_unit(op2):
                    def emit():
                        pp = p0 + op2 * 128
                        psv = ps.tile([128, BLK], dt.float32, tag=f"proj{op2 % 2}", name=f"vtp{op2}_{blk}")
                        for cc in range(2):
                            nc.tensor.matmul(
                                psv[:, :256],
                                lhsT=xt_sb[:, cc, pp : pp + 128],
                                rhs=wv_sb[:, cc],
                                start=(cc == 0), stop=(cc == 1),
                            )
                        nc.vector.tensor_copy(st["vt"][op2][:], psv[:, :256])
                    return emit

                def rope_unit(cht):
                    # cht1 rides the idle gpsimd; vector takes cht0
                    eng = nc.vector if cht == 0 else nc.gpsimd
                    def emit():
                        for h in range(2):
                            sl = slice(h * 1024, (h + 1) * 1024)
                            eng.tensor_mul(st["cq"][cht][:, sl], st["q"][cht][:, sl], ct_sb[:])
                            eng.tensor_mul(st["sq"][cht][:, sl], st["q"][cht][:, sl], st_sb[:])
                    return emit

                def kprod_unit(dh):
                    # rk[dh] = ke[dh]*ct +- ke[2+dh]*st; rpk[dh] = ke[dh]*st -+ ke[2+dh]*ct
                    def emit():
                        pk = sb.tile([128, BLK], dt.bfloat16, tag=f"pk_{r}", name=f"pk{dh}_{blk}")
                        psw = sb.tile([128, BLK], dt.bfloat16, tag=f"psw_{r}", name=f"psw{dh}_{blk}")
                        nc.vector.tensor_mul(pk[:], st["ke"][:, dh, :], ct_sb[:, :BLK])
                        nc.vector.tensor_mul(psw[:], st["ke"][:, 2 + dh, :], st_sb[:, :BLK])
                        if dh == 0:
                            nc.vector.tensor_add(st["rk"][dh][:], pk[:], psw[:])
                        else:
                            nc.vector.tensor_sub(st["rk"][dh][:], pk[:], psw[:])
                        pk2 = sb.tile([128, BLK], dt.bfloat16, tag=f"pk2_{r}", name=f"pk2{dh}_{blk}")
                        psw2 = sb.tile([128, BLK], dt.bfloat16, tag=f"psw2_{r}", name=f"psw2{dh}_{blk}")
                        nc.vector.tensor_mul(pk2[:], st["ke"][:, dh, :], st_sb[:, :BLK])
                        nc.vector.tensor_mul(psw2[:], st["ke"][:, 2 + dh, :], ct_sb[:, :BLK])
                        if dh == 0:
                            nc.vector.tensor_sub(st["rpk"][dh][:], pk2[:], psw2[:])
                        else:
                            nc.vector.tensor_add(st["rpk"][dh][:], pk2[:], psw2[:])
                    return emit

                # order separates proj-psum ring mates (alternating proj0/proj1
                # tags) so a slow cast never directly stalls the next PE matmul
                units = [k_unit(0), k_unit(1), q_unit(0, 0), q_unit(1, 0),
                         k_unit(2), k_unit(3), q_unit(2, 0), q_unit(3, 0),
                         kprod_unit(0), kprod_unit(1), rope_unit(0),
                         q_unit(0, 1), q_unit(1, 1), q_unit(2, 1), q_unit(3, 1),
                         rope_unit(1)]
                units += [vt_unit(i) for i in range(4)]
                return units

            def ham_unit(blk, n):
                # dependency-free matmuls chewed while the elementwise stage
                # catches up at a block boundary — keeps the HAM clock-gate warm
                def emit():
                    hp = ps.tile([128, 512], dt.float32, tag="proj0", name=f"ham_{blk}")
                    for _ in range(n):
                        nc.tensor.matmul(hp[0:128, 0:256], lhsT=junk[:, 0:128],
                                         rhs=junk[:, 0:256], start=True, stop=True)
                return emit

            def attn_units(blk):
                st = S[blk]
                units = []

                def qk_unit(op2p, quad):
                    def emit():
                        qkA = ps.tile([128, 1024], dt.float32, tag=f"qk{S['qk_i'] % 3}", name=f"qkA{op2p}_{quad}_{blk}")
                        S["qk_i"] += 1
                        qkB = ps.tile([128, 1024], dt.float32, tag=f"qk{S['qk_i'] % 3}", name=f"qkB{op2p}_{quad}_{blk}")
                        S["qk_i"] += 1
                        qkg = [qkA, qkA, qkB, qkB]
                        for g4 in range(4):
                            for op2l in range(2):
                                for oo in range(2):
                                    o = (op2p * 2 + op2l) * 2 + oo
                                    for ph, (kt, ut) in enumerate(((st["rk"], st["cq"]), (st["rpk"], st["sq"]))):
                                        foff = 512 * (g4 % 2) + 256 * op2l
                                        nc.tensor.matmul(
                                            qkg[g4][64 * oo : 64 * oo + 64, foff : foff + 256],
                                            lhsT=kt[quad][32 * g4 : 32 * g4 + 32, o * 64 : (o + 1) * 64],
                                            rhs=ut[quad][:]
                                            .rearrange("p (m t) -> p m t", m=M)[
                                                32 * g4 : 32 * g4 + 32, :, o * 64 : (o + 1) * 64
                                            ],
                                            start=(ph == 0), stop=(ph == 1),
                                            tile_position=(32 * g4, 64 * oo),
                                        )
                        w_sb = sb.tile([128, 2048], dt.bfloat16, tag=f"w{(op2p * 2 + quad) % 4}", name=f"w{op2p}_{quad}_{blk}")
                        nc.scalar.activation(w_sb[:, 0:1024], qkA[:], AF.Sigmoid, scale=SCALE)
                        nc.scalar.activation(w_sb[:, 1024:2048], qkB[:], AF.Sigmoid, scale=SCALE)
                        st[f"w{op2p}_{quad}"] = w_sb
                    return emit

                def av_unit(op2p, op2l):
                    def emit():
                        op2 = op2p * 2 + op2l
                        av = ps.tile([128, 1024], dt.float32, tag=f"qk{S['qk_i'] % 3}", name=f"av{op2}_{blk}")
                        S["qk_i"] += 1
                        for quad in range(2):
                            for oo in range(2):
                                for g4 in range(4):
                                    woff = 1024 * (g4 // 2) + 512 * (g4 % 2) + 256 * op2l
                                    nc.tensor.matmul(
                                        av[32 * g4 : 32 * g4 + 32,
                                           512 * oo + 256 * quad : 512 * oo + 256 * quad + 256],
                                        lhsT=st["vt"][op2][
                                            64 * oo : 64 * oo + 64,
                                            32 * (quad * 4 + g4) : 32 * (quad * 4 + g4) + 32,
                                        ],
                                        rhs=st[f"w{op2p}_{quad}"][64 * oo : 64 * oo + 64, woff : woff + 256],
                                        start=True, stop=True,
                                        tile_position=(64 * oo, 32 * g4),
                                    )
                        av4 = av[:].rearrange("p (oo q mt) -> p oo q mt", oo=2, q=2)
                        for quad in range(2):
                            nc.vector.tensor_copy(
                                st["vc"][quad][:, op2 * 512 : (op2 + 1) * 512]
                                .rearrange("p (oo mt) -> p oo mt", oo=2),
                                av4[:, :, quad, :],
                            )
                    return emit

                # all qk units first (their sigmoids pipeline on scalar),
                # then the av units, which need 4 sigmoids each completed
                if blk >= 1:
                    units.append(ham_unit(blk, 10))
                for op2p in range(2):
                    units.append(qk_unit(op2p, 0))
                    units.append(qk_unit(op2p, 1))
                units.append(av_unit(0, 0))
                units.append(av_unit(0, 1))
                units.append(av_unit(1, 0))
                units.append(av_unit(1, 1))
                return units

            def outproj_units(blk):
                st = S[blk]
                o_sb = sb.tile([128, 2, BLK], dt.bfloat16, tag=f"osb_{blk % 2}", name=f"osb_{blk}")

                def unit(och):
                    def emit():
                        pso = ps.tile([128, BLK], dt.float32, tag=f"proj{och % 2}", name=f"ops{och}_{blk}")
                        for ch in range(8):
                            m, quad = ch // 2, ch % 2
                            nc.tensor.matmul(
                                pso[:],
                                lhsT=wo_sb[:, ch, och * 128 : (och + 1) * 128],
                                rhs=st["vc"][quad][:]
                                .rearrange("p (o m t) -> p o m t", o=8, m=M)[:, :, m, :],
                                start=(ch == 0), stop=(ch == 7),
                            )
                        nc.scalar.copy(o_sb[:, och, :], pso[:])
                    return emit

                def dma_unit(och):
                    def emit():
                        nc.sync.dma_start(
                            out=outT[och * 128 : (och + 1) * 128, blk * BLK : (blk + 1) * BLK],
                            in_=o_sb[:, och, :],
                        )
                    return emit

                return [unit(0), dma_unit(0), unit(1), dma_unit(1)]

            def interleave(primary, fillers, front=5):
                """Emit primary units with fillers spread between them, with
                extra weight on the first `front` primaries (the qk phase) so
                the av-phase tail stays free of filler-induced PE stalls."""
                if not primary:
                    for f in fillers:
                        f()
                    return
                k = len(fillers)
                n = len(primary)
                w = [2 if i < front else 1 for i in range(n)]
                tot = sum(w)
                fi = 0
                acc = 0
                for i, p in enumerate(primary):
                    p()
                    acc += w[i]
                    take = (k * acc) // tot - fi
                    for _ in range(take):
                        fillers[fi]()
                        fi += 1

            S["qk_i"] = 0
            # prologue: block-0 projections, warming late tensors just in time
            p0units = proj_units(0)
            # unit order: 4 k, 2 kprod, 4 q, rope, 4 q, rope, 4 vt
            for i, u in enumerate(p0units):
                if i == 2:
                    warm_touch(wq_sb[:, 0, 0:8]); warm_touch(wq_sb[:, 1, 0:8])
                if i == 6:
                    warm_touch(wq_sb[:, 0, 512:520]); warm_touch(wq_sb[:, 1, 512:520])
                if i == 16:
                    warm_touch(wv_sb[:, 0, 0:8]); warm_touch(wv_sb[:, 1, 0:8])
                u()
            warm_touch(wo_sb[:, 0, 0:8])
            for blk in range(NBLK):
                fillers = []
                if blk + 1 < NBLK:
                    for cc in range(2):
                        warm_touch(xt_sb[:, cc, (blk + 1) * BLK : (blk + 1) * BLK + 8])
                    fillers += proj_units(blk + 1)
                if blk - 1 >= 0:
                    fillers += outproj_units(blk - 1)
                interleave(attn_units(blk), fillers)
            for u in outproj_units(NBLK - 1):
                u()

    nc.compile()
    return nc


# ---------------------------------------------------------------- host side
def _rope_tables(pos_arr, rope_freq):
    scaling = np.pi / np.stack([np.linspace(1, 30, HH), np.linspace(0.1, 1, HH)], -1)
    freq = rope_freq * scaling.astype(np.float32)
    phi = (pos_arr[:, None, :] * freq[None, :, :]).sum(-1)   # [64, HH]
    cs, sn = np.cos(phi), np.sin(phi)
    Ct = np.repeat(cs.T, 2, axis=0).astype(np.float32)        # [32, 64] rows h
    St = np.repeat(sn.T, 2, axis=0).astype(np.float32)
    return np.tile(Ct, (4, 16)), np.tile(St, (4, 16))         # [128, 1024]


def _build_weights(Wq, Wk, Wv, Wo, axis):
    Wq_a = Wq[:, 2 * axis : 2 * axis + 2]                     # [C, d, v, M, KG, H]
    wq = np.transpose(Wq_a, (0, 3, 1, 2, 4, 5)).reshape(C, 1024)  # (m, d, v, kg, h)
    Wk_a = Wk[:, 2 * axis : 2 * axis + 2].reshape(C, 2, 2, KG, HH, 2)
    Wk_swap = np.stack([Wk_a[..., 1], -Wk_a[..., 0]], -1)
    wk = np.concatenate([Wk_a.reshape(C, 256), Wk_swap.reshape(C, 256)], 1)  # (e,d,v,kg,h)
    wv = (Wv[:, 2 * axis : 2 * axis + 2].reshape(C, 256) * DEN).astype(np.float32)
    Wo_a = Wo[2 * axis : 2 * axis + 2]                        # [d, v, M, KG, HV, C]
    Wo_perm = np.transpose(Wo_a, (2, 0, 1, 3, 4, 5)).reshape(M * 256, C)  # (m, c, f)
    wo = Wo_perm.reshape(8, 128, 256).transpose(1, 0, 2).reshape(128, 8 * 256)
    import ml_dtypes
    return (np.ascontiguousarray(wq).astype(ml_dtypes.bfloat16),
            np.ascontiguousarray(wk).astype(ml_dtypes.bfloat16),
            np.ascontiguousarray(wv).astype(ml_dtypes.bfloat16),
            np.ascontiguousarray(wo).astype(ml_dtypes.bfloat16))


def prepare_in_maps(x, Wq, Wk, Wv, bv, Wo, rope_freq, ypos, xpos, mask):
    import ml_dtypes

    assert np.abs(bv).max() == 0.0, "kernel assumes bv == 0 (spec fill=zeros)"
    Ct0, St0 = _rope_tables(ypos, rope_freq)
    Ct1, St1 = _rope_tables(xpos, rope_freq)
    waxis = [_build_weights(Wq, Wk, Wv, Wo, a) for a in range(2)]
    in_maps = []
    for core in range(8):
        b, axis, half = core // 4, (core // 2) % 2, core % 2
        wq, wk, wv, wo = waxis[axis]
        if axis == 0:
            blkx = x[b, :, 32 * half : 32 * half + 32, :]     # [Y, 32, C]
            xT = np.transpose(blkx, (2, 1, 0)).reshape(C, NPOS)  # (c, o=x, t=y)
            Ct, St = Ct0, St0
        else:
            blkx = x[b, 32 * half : 32 * half + 32, :, :]     # [32, X, C]
            xT = np.transpose(blkx, (2, 0, 1)).reshape(C, NPOS)  # (c, o=y, t=x)
            Ct, St = Ct1, St1
        in_maps.append(
            dict(
                xT=np.ascontiguousarray(xT).astype(ml_dtypes.bfloat16),
                wq=wq, wk=wk, wv=wv, wo=wo,
                ctab=Ct.astype(ml_dtypes.bfloat16),
                stab=St.astype(ml_dtypes.bfloat16),
            )
        )
    return in_maps


def gather_output(results):
    out = np.zeros((B, Y, X, C), np.float32)
    for core in range(8):
        b, axis, half = core // 4, (core // 2) % 2, core % 2
        outT = np.asarray(results[core]["outT"], np.float32).reshape(C, 32, 64)
        if axis == 0:
            out[b, :, 32 * half : 32 * half + 32, :] += np.transpose(outT, (2, 1, 0))
        else:
            out[b, 32 * half : 32 * half + 32, :, :] += np.transpose(outT, (1, 2, 0))
    return out


_CACHED = {}


def kernel(x, Wq, Wk, Wv, bv, Wo, rope_freq, ypos, xpos, mask):
    from concourse.bass_utils import run_bass_kernel_spmd

    x, Wq, Wk, Wv, bv, Wo, rope_freq, ypos, xpos = (
        np.asarray(a, np.float32) for a in (x, Wq, Wk, Wv, bv, Wo, rope_freq, ypos, xpos)
    )
    in_maps = prepare_in_maps(x, Wq, Wk, Wv, bv, Wo, rope_freq, ypos, xpos, mask)
    if "nc" not in _CACHED:
        _CACHED["nc"] = build_program()
    res = run_bass_kernel_spmd(_CACHED["nc"], in_maps, core_ids=list(range(8)))
    return gather_output(res.results)
